# revision 1
# baseline (speedup 1.0000x reference)
"""AttentionPooling Trainium2 kernel.

Computes, for G=512 graphs over N=500000 nodes (batch sorted):
    s   = tanh(x @ W1 + b1) @ W2 + b2            # [N]
    w   = segment_softmax(s, batch)              # [N]
    out = segment_sum(x * w[:, None], batch)     # [G, 256]

Key observations:
  * |s| <= ||W2||_1 + |b2| ~ 11, so exp(s) never overflows fp32 and the
    segment-max subtraction in the reference softmax can be skipped
    entirely (softmax is shift-invariant).
  * out[g] = U[g] / Z[g] with U = sum_i e_i x_i, Z = sum_i e_i -- both are
    segment sums, computed on the TensorEngine as A_e^T @ [x | 1] where
    A_e[i, g] = e_i * (batch_i == g) is a weighted one-hot built in ONE
    DVE tensor_scalar op (is_equal then mult).

Sharding: 64 contiguous graphs per core (batch is sorted so node ranges are
contiguous). Each core is fully independent - no collectives.

Host prep: x is cast to bf16 and laid out twice (natural with a ones column
appended, and transposed for the score matmul). Per core that is ~65MB of
HBM traffic, i.e. the same bytes as reading x once in f32.
"""

import os

import ml_dtypes
import numpy as np

import concourse.bass as bass
import concourse.mybir as mybir
from concourse import bass_utils
from concourse.tile import TileContext

F32 = mybir.dt.float32
BF16 = mybir.dt.bfloat16
FP8 = mybir.dt.float8e4

N_NODES = 500000
HIDDEN = 256
N_GRAPHS = 512
N_CORES = 8
GPC = N_GRAPHS // N_CORES  # graphs per core = 64
SUPER = 8  # node-tiles (of 128) per group
GROUP = SUPER * 128  # 1024 nodes per group

LAST_RESULT = None  # BassKernelResults of the most recent run (for test.py)


def split_excess_waits(nc: bass.Bass) -> int:
    """Walrus in this toolchain accepts at most one sync-wait per instruction
    (two for EventSemaphore). Tile emits more; split the surplus into
    standalone EventSemaphore instructions ahead of the offender."""
    n_split = 0
    for f in nc.m.functions:
        for bb in f.blocks:
            new = []
            for ins in bb.instructions:
                si = ins.sync_info
                waits = list(si.on_wait) if (si and si.on_wait) else []
                cap = 2 if type(ins).__name__ == "InstEventSemaphore" else 1
                if len(waits) <= cap:
                    new.append(ins)
                    continue
                keep = waits[-cap:]
                extra = waits[:-cap]
                for i in range(0, len(extra), 2):
                    ev = mybir.InstEventSemaphore(
                        name=f"{ins.name}-aw{i}",
                        engine=ins.engine,
                        ins=[],
                        outs=[],
                        sync_info=mybir.SyncInfo(
                            on_wait=extra[i : i + 2], on_update=[]
                        ),
                    )
                    new.append(ev)
                    n_split += 1
                ins.sync_info = mybir.SyncInfo(
                    on_wait=keep,
                    on_update=list(si.on_update) if si.on_update else [],
                )
                new.append(ins)
            bb.instructions = new
    return n_split


def build_nc(n_pad: int, n_reps: int = 1, ablate: str = "") -> bass.Bass:
    ablates = set(ablate.split("+")) if ablate else set()
    T = n_pad // 128  # node tiles per core
    NG = T // SUPER  # groups per core
    nc = bass.Bass()

    NGg = n_pad // GROUP
    xaug = nc.dram_tensor("xaug", [128, n_pad // 128, 260], BF16, kind="ExternalInput")
    xt = nc.dram_tensor("xt", [128, NGg, 2, GROUP], BF16, kind="ExternalInput")
    relt = nc.dram_tensor("relt", [128, T], F32, kind="ExternalInput")
    iota = nc.dram_tensor("iota", [128, GPC], F32, kind="ExternalInput")
    w1 = nc.dram_tensor("w1", [256, 128], BF16, kind="ExternalInput")
    w2 = nc.dram_tensor("w2", [128, 1], BF16, kind="ExternalInput")
    b1 = nc.dram_tensor("b1", [128, 1], F32, kind="ExternalInput")
    b2 = nc.dram_tensor("b2", [128, 1], F32, kind="ExternalInput")
    out = nc.dram_tensor("out", [GPC, HIDDEN], F32, kind="ExternalOutput")

    with TileContext(nc) as tc:
        with (
            tc.tile_pool(name="consts", bufs=1) as cpool,
            tc.tile_pool(name="xt_pool", bufs=4) as xtpool,
            tc.tile_pool(name="xa_pool", bufs=4) as xapool,
            tc.tile_pool(name="th_pool", bufs=6) as thpool,
            tc.tile_pool(name="e_pool", bufs=12) as epool,
            tc.tile_pool(name="ae_pool", bufs=12) as aepool,
            tc.tile_pool(name="fin_pool", bufs=1) as finpool,
            tc.tile_pool(name="ps_h", bufs=2, space="PSUM") as psh,
            tc.tile_pool(name="ps_s", bufs=2, space="PSUM") as pss,
            tc.tile_pool(name="ps_u", bufs=1, space="PSUM") as psu,
        ):
            w1_sb = cpool.tile([128, 2, 128], BF16)
            nc.sync.dma_start(out=w1_sb, in_=w1[:, :].rearrange("(c p) m -> p c m", c=2))
            w2_sb = cpool.tile([128, 1], BF16)
            nc.sync.dma_start(out=w2_sb, in_=w2[:, :])
            b1_sb = cpool.tile([128, 1], F32)
            nc.sync.dma_start(out=b1_sb, in_=b1[:, :])
            b2_sb = cpool.tile([128, 1], F32)
            nc.sync.dma_start(out=b2_sb, in_=b2[:, :])
            rel_sb = cpool.tile([128, T], F32)
            nc.sync.dma_start(out=rel_sb, in_=relt[:, :])
            iota_sb = cpool.tile([128, GPC], F32)
            nc.sync.dma_start(out=iota_sb, in_=iota[:, :])

            for _rep in range(n_reps):
                u_ps = None
                if not (ablates & {"dma_only", "no_u"}):
                    u_ps = psu.tile([GPC, 257], F32)  # [:, :256]=U, [:, 256]=Z

                n_h = NG * 2  # pipeline phases of 4 node-tiles each
                xa_tiles = {}
                xt_tiles = {}
                th_tiles = {}
                ae_tiles = {}

                def ensure_group(g):
                    if g in xt_tiles or g >= NG:
                        return
                    if "no_dma" in ablates:
                        if "const" not in xt_tiles:
                            xt_c = xtpool.tile([128, 2, GROUP], BF16)
                            nc.sync.dma_start(out=xt_c, in_=xt[:, 0, :, :])
                            xa_c = xapool.tile([128, SUPER, 260], BF16)
                            nc.sync.dma_start(out=xa_c, in_=xaug[:, 0:SUPER, :])
                            xt_tiles["const"] = xt_c
                            xa_tiles["const"] = xa_c
                        xt_tiles[g] = xt_tiles["const"]
                        xa_tiles[g] = xa_tiles["const"]
                        return
                    xt_t = xtpool.tile([128, 2, GROUP], BF16)
                    nc.sync.dma_start(out=xt_t, in_=xt[:, g, :, :])
                    xa_t = xapool.tile([128, SUPER, 260], BF16)
                    nc.scalar.dma_start(
                        out=xa_t, in_=xaug[:, g * SUPER : (g + 1) * SUPER, :]
                    )
                    xt_tiles[g] = xt_t
                    xa_tiles[g] = xa_t

                def do_h_tanh(H):
                    g, hf = divmod(H, 2)
                    ensure_group(g)
                    ensure_group(g + 1)
                    ensure_group(g + 2)
                    hp = psh.tile([128, 512], F32)
                    for c in range(2):
                        nc.tensor.matmul(
                            hp,
                            lhsT=w1_sb[:, c, :],
                            rhs=xt_tiles[g][:, c, hf * 512 : (hf + 1) * 512],
                            start=(c == 0),
                            stop=(c == 1),
                        )
                    th = thpool.tile([128, 512], BF16)
                    nc.scalar.activation(
                        th, hp, mybir.ActivationFunctionType.Tanh, bias=b1_sb
                    )
                    th_tiles[H] = th

                def do_scores(H):
                    th = th_tiles.pop(H)
                    aes = []
                    for jj in range(4):
                        t = H * 4 + jj
                        sp = pss.tile([128, 1], F32)
                        nc.tensor.matmul(
                            sp, lhsT=th[:, jj * 128 : (jj + 1) * 128], rhs=w2_sb
                        )
                        e_sb = epool.tile([128, 1], F32)
                        nc.scalar.activation(
                            e_sb, sp, mybir.ActivationFunctionType.Exp, bias=b2_sb
                        )
                        ae = aepool.tile([128, GPC], BF16)
                        nc.vector.tensor_scalar(
                            ae,
                            iota_sb,
                            rel_sb[:, t : t + 1],
                            e_sb,
                            op0=mybir.AluOpType.is_equal,
                            op1=mybir.AluOpType.mult,
                        )
                        aes.append((t, ae))
                    ae_tiles[H] = aes

                def do_u(H):
                    g, hf = divmod(H, 2)
                    u_n = 128 if "small_u" in ablates else 257
                    for idx, (t, ae) in enumerate(ae_tiles.pop(H)):
                        j = hf * 4 + idx
                        nc.tensor.matmul(
                            u_ps[:, 0:u_n],
                            lhsT=ae,
                            rhs=xa_tiles[g][:, j, 0:u_n],
                            start=(t == 0),
                            stop=(t == T - 1),
                        )
                    if hf == 1:
                        del xa_tiles[g]

                if "dma_only" in ablates:
                    for g in range(NG):
                        ensure_group(g)
                else:
                    do_h_tanh(0)
                    for H in range(n_h):
                        if H + 1 < n_h:
                            do_h_tanh(H + 1)
                        do_scores(H)
                        if H > 0 and "no_u" not in ablates:
                            do_u(H - 1)
                    if "no_u" not in ablates:
                        do_u(n_h - 1)

                if ablates & {"dma_only", "no_u"}:
                    o_sb = finpool.tile([GPC, HIDDEN], F32)
                    nc.vector.memset(o_sb, 0.0)
                    nc.sync.dma_start(out=out[:, :], in_=o_sb)
                else:
                    z_sb = finpool.tile([GPC, 1], F32)
                    nc.vector.tensor_scalar_max(z_sb, u_ps[:, 256:257], 1e-30)
                    rz_sb = finpool.tile([GPC, 1], F32)
                    nc.vector.reciprocal(rz_sb, z_sb)
                    o_sb = finpool.tile([GPC, HIDDEN], F32)
                    nc.vector.tensor_scalar_mul(o_sb, u_ps[:, 0:256], rz_sb)
                    nc.sync.dma_start(out=out[:, :], in_=o_sb)

    split_excess_waits(nc)
    return nc


def kernel(x, batch, W1, b1, W2, b2):
    global LAST_RESULT
    x = np.asarray(x, dtype=np.float32)
    batch = np.asarray(batch)
    W1 = np.asarray(W1, dtype=np.float32)
    b1 = np.asarray(b1, dtype=np.float32)
    W2 = np.asarray(W2, dtype=np.float32)
    b2 = np.asarray(b2, dtype=np.float32)

    # per-core contiguous graph ranges (batch is sorted)
    bounds = np.searchsorted(batch, np.arange(0, N_GRAPHS + 1, GPC))
    n_per_core = np.diff(bounds)
    n_pad = int(-(-n_per_core.max() // GROUP) * GROUP)
    t_tiles = n_pad // 128

    xbf = x.astype(ml_dtypes.bfloat16)
    w1bf = W1.astype(ml_dtypes.bfloat16)
    w2bf = W2.reshape(128, 1).astype(ml_dtypes.bfloat16)
    b1c = np.ascontiguousarray(b1.reshape(128, 1), dtype=np.float32)
    b2c = np.full((128, 1), np.float32(b2.reshape(-1)[0]), dtype=np.float32)
    iota_bc = np.ascontiguousarray(
        np.broadcast_to(np.arange(GPC, dtype=np.float32), (128, GPC))
    )

    in_maps = []
    for k in range(N_CORES):
        s, e = int(bounds[k]), int(bounds[k + 1])
        nk = e - s
        xaug_flat = np.zeros((n_pad, 260), dtype=ml_dtypes.bfloat16)
        xaug_flat[:nk, :256] = xbf[s:e]
        xaug_flat[:nk, 256] = 1.0
        # [128, T, 260]: partition-major so each group DMA is one
        # contiguous run per partition
        xaug_k = np.ascontiguousarray(
            xaug_flat.reshape(t_tiles, 128, 260).transpose(1, 0, 2)
        )
        xpad = np.zeros((n_pad, 256), dtype=ml_dtypes.bfloat16)
        xpad[:nk] = xbf[s:e]
        ng = n_pad // GROUP
        # [128, NG, 2, GROUP]: xt_k[p, g, c, n] = x[g*GROUP+n, c*128+p]
        xt_k = np.ascontiguousarray(
            xpad.reshape(ng, GROUP, 2, 128).transpose(3, 0, 2, 1)
        )
        rel = np.full(n_pad, -1.0, dtype=np.float32)
        rel[:nk] = (batch[s:e] - k * GPC).astype(np.float32)
        relt_k = np.ascontiguousarray(rel.reshape(t_tiles, 128).T)
        in_maps.append(
            {
                "xaug": xaug_k,
                "xt": xt_k,
                "relt": relt_k,
                "iota": iota_bc,
                "w1": w1bf,
                "w2": w2bf,
                "b1": b1c,
                "b2": b2c,
            }
        )

    nc = build_nc(n_pad)
    LAST_RESULT = bass_utils.run_bass_kernel_spmd(
        nc,
        in_maps,
        core_ids=list(range(N_CORES)),
        trace=bool(int(os.environ.get("ATTN_TRACE", "0"))),
    )
    out = np.concatenate([r["out"] for r in LAST_RESULT.results], axis=0)
    return np.ascontiguousarray(out, dtype=np.float32)

